# revision 32
# baseline (speedup 1.0000x reference)
# Sliding-window causal multi-head attention with RoPE for Trainium2.
#
# Problem: B=4, T=2048, D=1024, H=16 heads, d_k=64, window=512.
#   q,k,v = x @ W{q,k,v}^T (split heads), RoPE(q,k), scores = q k^T / 8 with
#   mask 0 <= i-j <= 512, softmax, out = (attn @ v) concat-heads @ Wo^T.
#
# Sharding: 8 cores = (batch b in 0..3) x (head-group of 8 heads). Each core
# runs the full T=2048 sequence for its 8 heads and produces a PARTIAL output
# projection (contraction over its 512 attn dims); the host sums the two
# head-group partials per batch. Head split avoids the K/V window-overlap
# recompute and the zero-pad softmax correction a sequence split needs.
#
# Host-side prep: x and all weights are cast to bf16 and pre-transposed into
# the exact SBUF layouts the PE consumes (m-major lhsT tiles), so the device
# does plain contiguous DMA loads only — no SWDGE casts, no xbar transposes.
# Wq/Wk rows are eo-permuted per head so RoPE's rotate-half is a 32-row group
# swap (PE permutation matmul), as in cs336 rope with (evens|odds) packing.
#
# On-chip pipeline (all matmuls bf16 with fp32 PSUM accumulation):
#   - Q^T/K^T projections produce [128 = 2 heads x (evens|odds), t] tiles;
#     RoPE via host cos/sin tables + pswap permutation matmul.
#   - scores are computed transposed, S^T[kv, q] = K Q^T, per (head, kv
#     block) over the 5-block sliding window span; exp on ACT (scale=1/8
#     folded in); boundary masks applied multiplicatively post-exp on
#     gpsimd (otherwise idle).
#   - PV uses a two-segment lhsT AP [ones | V_h] so one matmul yields the
#     softmax denominator (rows 0:64) AND O^T (rows 64:128); normalization
#     is reciprocal_approx_fast + multiply into bf16 attnT tiles.
#   - scores of sub-step s are software-pipelined against PV of s-1 and the
#     next pair's K projection so the in-order PE queue never starves while
#     ACT drains the exp chain.

import dataclasses
from contextlib import ExitStack

import numpy as np
import ml_dtypes

BF16 = ml_dtypes.bfloat16

B, T, D = 4, 2048, 1024
H, DK = 16, 64
WIN = 512
THETA = 10000.0
NBT = T // 128  # 16 t/kv blocks
NCH = D // 128  # 8 contraction chunks
NPAIR = 4  # head pairs per core

_CACHE = {}


def _pair_cols(ap2d, a, b, w):
    """From a [P, F] AP over contiguous cols, build an AP over cols
    {a..a+w} then {b..b+w} (2D free: outer count 2 step b-a)."""
    base = ap2d[:, a : a + w]
    return dataclasses.replace(base, ap=[base.ap[0], [b - a, 2], [1, w]])


def _build(debug_dumps=False):
    import concourse.bass as bass
    import concourse.bacc as bacc
    import concourse.mybir as mybir
    import concourse.tile as tile

    dt = mybir.dt
    F32, BF = dt.float32, dt.bfloat16
    AF = mybir.ActivationFunctionType
    OP = mybir.AluOpType

    nc = bacc.Bacc("TRN2", target_bir_lowering=False, debug=False, num_devices=8)

    # ---- DRAM I/O (all device inputs are host-prepped bf16 layouts) ----
    xt_in = nc.dram_tensor("xt", [128, 4 * NCH * 512], BF, kind="ExternalInput").ap()
    wq_in = nc.dram_tensor("wq", [128, NCH * 512], BF, kind="ExternalInput").ap()
    wk_in = nc.dram_tensor("wk", [128, NCH * 512], BF, kind="ExternalInput").ap()
    wv_in = nc.dram_tensor("wv", [128, NCH * 512], BF, kind="ExternalInput").ap()
    wo_in = nc.dram_tensor("wo", [128, 4 * 1024], BF, kind="ExternalInput").ap()
    # csin = per-tch [cos(512) | sin(512)]; pk = [pswap(128) | masks(256)]
    csin_in = nc.dram_tensor("csin", [128, 4 * 2 * 512], BF, kind="ExternalInput").ap()
    pk_in = nc.dram_tensor("pk", [128, 384], BF, kind="ExternalInput").ap()
    sign_in = nc.dram_tensor("sign_t", [128, 1], F32, kind="ExternalInput").ap()
    # partial output in bf16 (host sums the two head-group partials in f32)
    out_d = nc.dram_tensor("out", [T, D], BF, kind="ExternalOutput").ap()

    with ExitStack() as ctx:
        tc = ctx.enter_context(tile.TileContext(nc))

        big = ctx.enter_context(tc.tile_pool(name="big", bufs=1))
        ab = ctx.enter_context(tc.tile_pool(name="ab", bufs=4))
        epool = ctx.enter_context(tc.tile_pool(name="epool", bufs=24))
        rpool = ctx.enter_context(tc.tile_pool(name="rpool", bufs=2))
        stpool = ctx.enter_context(tc.tile_pool(name="stpool", bufs=2))
        # PSUM (8 banks): proj/swap/Wo 2x1 + scores 2x2 + pv 2x1
        mmps = ctx.enter_context(tc.tile_pool(name="mmps", bufs=2, space="PSUM"))
        scps = ctx.enter_context(tc.tile_pool(name="scps", bufs=2, space="PSUM"))
        pvps = ctx.enter_context(tc.tile_pool(name="pvps", bufs=2, space="PSUM"))

        # ---- persistent SBUF ----
        xT = big.tile([128, 4, NCH, 512], BF)  # [m-part, tch, chunk, t]
        qT = big.tile([128, NPAIR, T], BF)
        kT = big.tile([128, NPAIR, T], BF)
        # per (kv block, head): [ones(64) | V_h(64)] so one PV matmul yields
        # the softmax denominator (out rows 0:64) and O^T (rows 64:128)
        vOnes = big.tile([128, NBT, 8, 128], BF)
        attnT = big.tile([128, NPAIR, T], BF)
        wqS = big.tile([128, NCH, 512], BF)
        wkS = big.tile([128, NCH, 512], BF)
        wvS = big.tile([128, NCH, 512], BF)
        woS = big.tile([128, 4, 1024], BF)
        csinS = big.tile([128, 4, 2, 512], BF)
        pkS = big.tile([128, 384], BF)
        signS = big.tile([128, 1], F32)
        pswapS = pkS[:, 0:128]
        maskS = pkS[:, 128:384]

        # ---- input DMAs: every tensor split in half across the two HWDGE
        # queues, emitted in strict first-use order so the first Q/V tiles
        # can start ~4us in instead of waiting behind monolithic loads ----
        nc.vector.memset(vOnes[:, :, :, 0:64], 1.0)

        def dma2(dst_lo, src_lo, dst_hi, src_hi):
            nc.scalar.dma_start(out=dst_lo, in_=src_lo)
            nc.sync.dma_start(out=dst_hi, in_=src_hi)

        def dma_w8(dst, src):  # [128, 8, 512] weight halves
            dma2(dst[:, 0:4, :], src[:, 0:2048], dst[:, 4:8, :], src[:, 2048:4096])

        def dma_csin(tch):
            c0 = tch * 1024
            dma2(
                csinS[:, tch, 0, :], csin_in[:, c0 : c0 + 512],
                csinS[:, tch, 1, :], csin_in[:, c0 + 512 : c0 + 1024],
            )

        def dma_xt(tch):
            c0 = tch * NCH * 512
            dma2(
                xT[:, tch, 0:4, :], xt_in[:, c0 : c0 + 2048],
                xT[:, tch, 4:8, :], xt_in[:, c0 + 2048 : c0 + 4096],
            )

        # tiny warmup DMAs: the DMA subsystem delivers ~22GB/s for its first
        # ~10us from cold; these absorb the ramp before the real loads
        warm = big.tile([128, 64], BF)
        nc.scalar.dma_start(out=warm[:, 0:32], in_=xt_in[:, 0:32])
        nc.sync.dma_start(out=warm[:, 32:64], in_=xt_in[:, 32:64])

        # first loads at 2-chunk granularity: chunk c's proj matmul can
        # start as soon as wq[c]+xt0[c] land instead of waiting 0.5MB halves
        for g in range(4):
            cs, ce = g * 1024, g * 1024 + 1024
            eng = nc.scalar if g % 2 == 0 else nc.sync
            eng.dma_start(out=wqS[:, 2 * g : 2 * g + 2, :], in_=wq_in[:, cs:ce])
            eng2 = nc.sync if g % 2 == 0 else nc.scalar
            eng2.dma_start(out=xT[:, 0, 2 * g : 2 * g + 2, :], in_=xt_in[:, cs:ce])
        nc.scalar.dma_start(out=signS, in_=sign_in)
        nc.sync.dma_start(out=pkS, in_=pk_in)
        dma_csin(0)
        dma_w8(wvS, wv_in)
        dma_xt(1)
        dma_csin(1)
        dma_csin(2)
        dma_w8(wkS, wk_in)
        dma_xt(2)
        dma_xt(3)
        dma_csin(3)
        dma2(woS[:, 0:2, :], wo_in[:, 0:2048], woS[:, 2:4, :], wo_in[:, 2048:4096])

        _evac_alt = [0]

        def _evac(out, in_):
            # alternate psum evacuations between ACT and DVE queues
            _evac_alt[0] ^= 1
            if _evac_alt[0]:
                nc.scalar.copy(out=out, in_=in_)
            else:
                nc.vector.tensor_copy(out, in_)

        # ---- projection tile helpers (split so the swap matmul can be
        # queued late, after other PE work, hiding the rope DVE latency) ----
        def proj_mm_r(wS, r, tch):
            ps = mmps.tile([128, 512], F32, tag="mm")
            for c in range(NCH):
                nc.tensor.matmul(
                    ps,
                    wS[:, c, r * 128 : r * 128 + 128],
                    xT[:, tch, c, :],
                    start=(c == 0),
                    stop=(c == NCH - 1),
                )
            return ps

        def rope_pre(ps, tch, evac_eng=None):
            pb = ab.tile([128, 512], BF, tag="pb")
            if evac_eng is None:
                _evac(pb, ps)
            elif evac_eng == "v":
                nc.vector.tensor_copy(pb, ps)
            w1 = ab.tile([128, 512], BF, tag="w1")
            t2 = ab.tile([128, 512], BF, tag="t2")
            nc.vector.tensor_mul(w1, pb, csinS[:, tch, 1, :])
            nc.vector.tensor_mul(t2, pb, csinS[:, tch, 0, :])
            return w1, t2

        def rope_swap(w1, t2, dest, r, tch, us_pool=None):
            tsl = slice(tch * 512, tch * 512 + 512)
            # phase 2 routes the swap psum to the (then-idle) pv pool so the
            # mmps ring holds one tile per proj and the PE never waits evac
            us = (us_pool or mmps).tile(
                [128, 512], F32, tag="pv" if us_pool is not None else "mm"
            )
            nc.tensor.matmul(us, pswapS, w1, start=True, stop=True)
            # rope = swap(P*sin) * sign + P*cos
            nc.vector.scalar_tensor_tensor(
                out=dest[:, r, tsl],
                in0=us,
                scalar=signS[:, 0:1],
                in1=t2,
                op0=OP.mult,
                op1=OP.add,
            )

        def v_tile(tt):
            tch, off = tt // 4, (tt % 4) * 128
            ps = mmps.tile([128, 512], F32, tag="mm")
            for c in range(NCH):
                nc.tensor.matmul(
                    ps,
                    xT[:, tch, c, off : off + 128],
                    wvS[:, c, :],
                    start=(c == 0),
                    stop=(c == NCH - 1),
                )
            _evac(vOnes[:, tt, :, 64:128], ps)

        # ---- phase 2: Q projection interleaved with V so the PE queue has
        # V work to fill Q's rope bubbles ----
        qlist = [(r, tch) for tch in range(4) for r in range(NPAIR)]  # 16
        for i in range(16):
            r, tch = qlist[i]
            ps = proj_mm_r(wqS, r, tch)
            w1, t2 = rope_pre(ps, tch)
            v_tile(i)
            rope_swap(w1, t2, qT, r, tch, us_pool=pvps)

        # ---- phase 3: K projection + attention, software-pipelined ----
        # sub-step si = 2p + sub. Scores of si interleave (in the PE queue)
        # with PV groups of si-1 and the next pair's K-projection chunks so
        # the PE keeps busy while ACT drains the per-block exp chain.
        e_tiles = {}

        def k_chunks(p):
            # 8 thunks: 4 mm chains and 4 swap finishes, swap_i after mm_i
            thunks = []
            pend = {}

            def mk_mm(tch):
                def f():
                    ps = proj_mm_r(wkS, p, tch)
                    pend[tch] = rope_pre(ps, tch, evac_eng="v")

                return f

            def mk_swap(tch):
                def f():
                    w1, t2 = pend.pop(tch)
                    rope_swap(w1, t2, kT, p, tch)

                return f

            order = [mk_mm(0), mk_mm(1), mk_swap(0), mk_mm(2), mk_swap(1),
                     mk_mm(3), mk_swap(2), mk_swap(3)]
            return order

        def sc_chunks(p, sub, si):
            rows = slice(64 * sub, 64 * sub + 64)

            def mk(b):
                def f():
                    ghi = min(b + 4, NBT - 1)
                    span = (ghi - b + 1) * 128
                    q0 = b * 128
                    sc = scps.tile([128, 640], F32, tag="sc")
                    for c0 in range(0, span, 512):
                        c1 = min(c0 + 512, span)
                        nc.tensor.matmul(
                            sc[:, c0:c1],
                            kT[rows, p, b * 128 : b * 128 + 128],
                            qT[rows, p, q0 + c0 : q0 + c1],
                            start=True,
                            stop=True,
                        )
                    et = epool.tile([128, 640], BF, tag="et")
                    nc.scalar.activation(
                        out=et[:, 0:span], in_=sc[:, 0:span], func=AF.Exp, scale=0.125
                    )
                    # boundary masks (multiplicative, post-exp) on gpsimd:
                    # causal at cols 0:128 (g=b), window at span-128 (g=b+4)
                    if b <= NBT - 5:
                        sel = _pair_cols(et[:, 0:640], 0, span - 128, 128)
                        nc.gpsimd.tensor_mul(sel, sel, maskS[:, 0:256])
                    else:
                        nc.gpsimd.tensor_mul(
                            et[:, 0:128], et[:, 0:128], maskS[:, 0:128]
                        )
                    e_tiles[(si, b)] = (et, q0, span)

                return f

            return [mk(b) for b in range(NBT)]

        def pv_chunks(p, sub, si):
            h = 2 * p + sub
            rows_h = slice(64 * sub, 64 * sub + 64)

            def mk(qh):
                def f():
                    qa0, qb0 = qh * 512, qh * 512 + 512
                    bstar = max(0, 4 * qh - 1)
                    blist = [bstar] + [
                        b
                        for b in range(NBT)
                        if b != bstar
                        and b * 128 < qb0
                        and (min(b + 4, NBT - 1) + 1) * 128 > qa0
                    ]
                    pv = pvps.tile([128, 512], F32, tag="pv")
                    for i, b in enumerate(blist):
                        et, q0, span = e_tiles[(si, b)]
                        ghi = min(b + 4, NBT - 1)
                        s0 = max(b * 128, qa0)
                        s1 = min((ghi + 1) * 128, qb0)
                        nc.tensor.matmul(
                            pv[:, s0 - qa0 : s1 - qa0],
                            vOnes[:, b, h, :],
                            et[:, s0 - q0 : s1 - q0],
                            start=(i == 0),
                            stop=(i == len(blist) - 1),
                        )
                    # normalize: attnT[rows_h] = O / sums
                    rec = rpool.tile([64, 512], F32, tag="rec")
                    nc.vector.reciprocal_approx_fast(out=rec, in_=pv[0:64, :])
                    nc.vector.tensor_mul(
                        attnT[rows_h, p, qa0:qb0], pv[64:128, :], rec
                    )

                return f

            return [mk(qh) for qh in range(4)]

        def zip_emit(primary, fillers):
            # emit primary thunks with fillers spliced in every 2 primaries
            fi = 0
            for i, th in enumerate(primary):
                th()
                if i % 2 == 1 and fi < len(fillers):
                    fillers[fi]()
                    fi += 1
            for th in fillers[fi:]:
                th()

        for th in k_chunks(0):
            th()
        prev_pv = None
        for p in range(NPAIR):
            kc = k_chunks(p + 1) if p < NPAIR - 1 else []
            for sub in range(2):
                si = 2 * p + sub
                fillers = []
                if prev_pv is not None:
                    fillers += prev_pv
                fillers += kc[:4] if sub == 0 else kc[4:]
                zip_emit(sc_chunks(p, sub, si), fillers)
                prev_pv = pv_chunks(p, sub, si)

        # ---- phase 4: output projection (partial: contraction over the 8
        # local heads; host sums the two head-group partials per batch),
        # interleaved with the last sub's PV groups ----
        def wo_tile(qt):
            st = stpool.tile([128, 1024], BF, tag="st")
            for nh in range(2):
                # nh1 psum lives in the (now idle) scores pool so the mmps
                # ring holds one tile per qt and the PE never waits evac
                if nh == 0:
                    ps = mmps.tile([128, 512], F32, tag="mm")
                else:
                    ps640 = scps.tile([128, 640], F32, tag="sc")
                    ps = ps640[:, 0:512]
                for c in range(NPAIR):
                    nc.tensor.matmul(
                        ps,
                        attnT[:, c, qt * 128 : qt * 128 + 128],
                        woS[:, c, nh * 512 : nh * 512 + 512],
                        start=(c == 0),
                        stop=(c == NPAIR - 1),
                    )
                _evac(st[:, nh * 512 : nh * 512 + 512], ps)
                eng = nc.sync if (qt + nh) % 2 == 0 else nc.scalar
                eng.dma_start(
                    out=out_d[qt * 128 : qt * 128 + 128, nh * 512 : nh * 512 + 512],
                    in_=st[:, nh * 512 : nh * 512 + 512],
                )

        for qh in range(4):
            prev_pv[qh]()
            for qt in range(4 * qh, 4 * qh + 4):
                wo_tile(qt)

        if debug_dumps:
            for nm, tl, sh in (
                ("d_xT", xT, [128, 4 * NCH * 512]),
                ("d_qT", qT, [128, NPAIR * T]),
                ("d_kT", kT, [128, NPAIR * T]),
                ("d_vOnes", vOnes, [128, NBT * 8 * 128]),
                ("d_attnT", attnT, [128, NPAIR * T]),
            ):
                dd = nc.dram_tensor(nm, sh, BF, kind="ExternalOutput").ap()
                nc.sync.dma_start(out=dd, in_=tl)

    nc.compile()
    return nc


def _host_inputs(x, token_positions, Wq, Wk, Wv, Wo):
    x = np.asarray(x, dtype=np.float32)
    pos = np.asarray(token_positions).astype(np.float64)
    Wq = np.asarray(Wq, np.float32)
    Wk = np.asarray(Wk, np.float32)
    Wv = np.asarray(Wv, np.float32)
    Wo = np.asarray(Wo, np.float32)

    invf = THETA ** (-np.arange(32, dtype=np.float64) / 32.0)
    ang = invf[:, None] * pos[None, :]  # [32, T]
    cos_t = np.tile(np.cos(ang), (4, 1)).astype(BF16)  # [128, T]
    sin_t = np.tile(np.sin(ang), (4, 1)).astype(BF16)
    # per-tch [cos(512) | sin(512)] chunks for just-in-time DMA
    csin = np.stack(
        [cos_t.reshape(128, 4, 512), sin_t.reshape(128, 4, 512)], axis=2
    )  # [128, 4, 2, 512]
    csin = np.ascontiguousarray(csin).reshape(128, 4096)
    perm = np.r_[32:64, 0:32, 96:128, 64:96]
    P = np.zeros((128, 128), np.float32)
    P[np.arange(128), perm] = 1.0
    pswapT = np.ascontiguousarray(P.T).astype(BF16)
    r = np.arange(128)[:, None]  # kv row (partition)
    c = np.arange(128)[None, :]  # q col (free)
    m_causal = (c >= r).astype(BF16)
    m_window = (c <= r).astype(BF16)
    pk = np.ascontiguousarray(
        np.concatenate([pswapT, m_causal, m_window], axis=1)
    )  # [128, 384]
    sign = np.tile(np.repeat(np.float32([-1, 1]), 32), 2).reshape(128, 1)

    def prep_qk(W, hg):
        Ws = W[hg * 512 : (hg + 1) * 512]  # [512 n, 1024 m]
        # eo-permute within each head: [64] -> [evens(32), odds(32)]
        Wp = Ws.reshape(8, 32, 2, 1024).transpose(0, 2, 1, 3).reshape(512, 1024)
        wt = Wp.T.reshape(NCH, 128, 512).transpose(1, 0, 2)  # [128, c, n']
        return np.ascontiguousarray(wt.astype(BF16)).reshape(128, NCH * 512)

    def prep_v(W, hg):
        Ws = W[hg * 512 : (hg + 1) * 512]
        wt = Ws.T.reshape(NCH, 128, 512).transpose(1, 0, 2)
        return np.ascontiguousarray(wt.astype(BF16)).reshape(128, NCH * 512)

    def prep_o(W, hg):
        WoC = W[:, hg * 512 : (hg + 1) * 512]  # [1024 n, 512 m]
        wt = WoC.T.reshape(4, 128, 1024).transpose(1, 0, 2)  # [128, c, n]
        return np.ascontiguousarray(wt.astype(BF16)).reshape(128, 4 * 1024)

    wq_hg = [prep_qk(Wq, hg) for hg in range(2)]
    wk_hg = [prep_qk(Wk, hg) for hg in range(2)]
    wv_hg = [prep_v(Wv, hg) for hg in range(2)]
    wo_hg = [prep_o(Wo, hg) for hg in range(2)]

    xt_b = []
    for b in range(B):
        xb = x[b].astype(BF16)  # [T, D]
        xt = xb.reshape(4, 512, NCH, 128).transpose(3, 0, 2, 1)  # [128,tch,c,j]
        xt_b.append(np.ascontiguousarray(xt).reshape(128, 4 * NCH * 512))

    in_maps = []
    for core in range(8):
        b, hg = divmod(core, 2)
        in_maps.append(
            {
                "xt": xt_b[b],
                "wq": wq_hg[hg],
                "wk": wk_hg[hg],
                "wv": wv_hg[hg],
                "wo": wo_hg[hg],
                "csin": csin,
                "pk": pk,
                "sign_t": sign,
            }
        )
    return in_maps


def _get_nc():
    if "nc" not in _CACHE:
        _CACHE["nc"] = _build()
    return _CACHE["nc"]


def kernel(x, token_positions, Wq, Wk, Wv, Wo, _trace=False):
    from concourse.bass_utils import run_bass_kernel_spmd

    nc = _get_nc()
    in_maps = _host_inputs(x, token_positions, Wq, Wk, Wv, Wo)
    res = run_bass_kernel_spmd(nc, in_maps, core_ids=list(range(8)), trace=_trace)
    _CACHE["last_result"] = res
    out = np.zeros((B, T, D), np.float32)
    for b in range(B):
        out[b] = res.results[2 * b]["out"].astype(np.float32) + res.results[
            2 * b + 1
        ]["out"].astype(np.float32)
    return out


# revision 33
# speedup vs baseline: 1.0716x; 1.0716x over previous
# Sliding-window causal multi-head attention with RoPE for Trainium2.
#
# Problem: B=4, T=2048, D=1024, H=16 heads, d_k=64, window=512.
#   q,k,v = x @ W{q,k,v}^T (split heads), RoPE(q,k), scores = q k^T / 8 with
#   mask 0 <= i-j <= 512, softmax, out = (attn @ v) concat-heads @ Wo^T.
#
# Sharding: 8 cores = (batch b in 0..3) x (head-group of 8 heads). Each core
# runs the full T=2048 sequence for its 8 heads and produces a PARTIAL output
# projection (contraction over its 512 attn dims); the host sums the two
# head-group partials per batch. Head split avoids the K/V window-overlap
# recompute and the zero-pad softmax correction a sequence split needs.
#
# Host-side prep: x and all weights are cast to bf16 and pre-transposed into
# the exact SBUF layouts the PE consumes (m-major lhsT tiles), so the device
# does plain contiguous DMA loads only — no SWDGE casts, no xbar transposes.
# Wq/Wk rows are eo-permuted per head so RoPE's rotate-half is a 32-row group
# swap (PE permutation matmul), as in cs336 rope with (evens|odds) packing.
#
# On-chip pipeline (all matmuls bf16 with fp32 PSUM accumulation):
#   - Q^T/K^T projections produce [128 = 2 heads x (evens|odds), t] tiles;
#     RoPE via host cos/sin tables + pswap permutation matmul.
#   - scores are computed transposed, S^T[kv, q] = K Q^T, per (head, kv
#     block) over the 5-block sliding window span; exp on ACT (scale=1/8
#     folded in); boundary masks applied multiplicatively post-exp on
#     gpsimd (otherwise idle).
#   - PV uses a two-segment lhsT AP [ones | V_h] so one matmul yields the
#     softmax denominator (rows 0:64) AND O^T (rows 64:128); normalization
#     is reciprocal_approx_fast + multiply into bf16 attnT tiles.
#   - scores of sub-step s are software-pipelined against PV of s-1 and the
#     next pair's K projection so the in-order PE queue never starves while
#     ACT drains the exp chain.

import dataclasses
from contextlib import ExitStack

import numpy as np
import ml_dtypes

BF16 = ml_dtypes.bfloat16

B, T, D = 4, 2048, 1024
H, DK = 16, 64
WIN = 512
THETA = 10000.0
NBT = T // 128  # 16 t/kv blocks
NCH = D // 128  # 8 contraction chunks
NPAIR = 4  # head pairs per core

_CACHE = {}


def _pair_cols(ap2d, a, b, w):
    """From a [P, F] AP over contiguous cols, build an AP over cols
    {a..a+w} then {b..b+w} (2D free: outer count 2 step b-a)."""
    base = ap2d[:, a : a + w]
    return dataclasses.replace(base, ap=[base.ap[0], [b - a, 2], [1, w]])


def _build(debug_dumps=False):
    import concourse.bass as bass
    import concourse.bacc as bacc
    import concourse.mybir as mybir
    import concourse.tile as tile

    dt = mybir.dt
    F32, BF = dt.float32, dt.bfloat16
    AF = mybir.ActivationFunctionType
    OP = mybir.AluOpType

    nc = bacc.Bacc("TRN2", target_bir_lowering=False, debug=False, num_devices=8)

    # ---- DRAM I/O (all device inputs are host-prepped bf16 layouts) ----
    xt_in = nc.dram_tensor("xt", [128, 4 * NCH * 512], BF, kind="ExternalInput").ap()
    wq_in = nc.dram_tensor("wq", [128, NCH * 512], BF, kind="ExternalInput").ap()
    wk_in = nc.dram_tensor("wk", [128, NCH * 512], BF, kind="ExternalInput").ap()
    wv_in = nc.dram_tensor("wv", [128, NCH * 512], BF, kind="ExternalInput").ap()
    wo_in = nc.dram_tensor("wo", [128, 4 * 1024], BF, kind="ExternalInput").ap()
    # csin = per-tch [cos(512) | sin(512)]; pk = [pswap(128) | masks(256)]
    csin_in = nc.dram_tensor("csin", [128, 4 * 2 * 512], BF, kind="ExternalInput").ap()
    pk_in = nc.dram_tensor("pk", [128, 384], BF, kind="ExternalInput").ap()
    sign_in = nc.dram_tensor("sign_t", [128, 1], F32, kind="ExternalInput").ap()
    # partial output in bf16 (host sums the two head-group partials in f32)
    out_d = nc.dram_tensor("out", [T, D], BF, kind="ExternalOutput").ap()

    with ExitStack() as ctx:
        tc = ctx.enter_context(tile.TileContext(nc))

        big = ctx.enter_context(tc.tile_pool(name="big", bufs=1))
        ab = ctx.enter_context(tc.tile_pool(name="ab", bufs=4))
        epool = ctx.enter_context(tc.tile_pool(name="epool", bufs=24))
        rpool = ctx.enter_context(tc.tile_pool(name="rpool", bufs=2))
        stpool = ctx.enter_context(tc.tile_pool(name="stpool", bufs=2))
        # PSUM (8 banks): proj/swap/Wo 2x1 + scores 2x2 + pv 2x1
        mmps = ctx.enter_context(tc.tile_pool(name="mmps", bufs=2, space="PSUM"))
        scps = ctx.enter_context(tc.tile_pool(name="scps", bufs=2, space="PSUM"))
        pvps = ctx.enter_context(tc.tile_pool(name="pvps", bufs=2, space="PSUM"))

        # ---- persistent SBUF ----
        xT = big.tile([128, 4, NCH, 512], BF)  # [m-part, tch, chunk, t]
        qT = big.tile([128, NPAIR, T], BF)
        kT = big.tile([128, NPAIR, T], BF)
        # per (kv block, head): [ones(64) | V_h(64)] so one PV matmul yields
        # the softmax denominator (out rows 0:64) and O^T (rows 64:128)
        vOnes = big.tile([128, NBT, 8, 128], BF)
        attnT = big.tile([128, NPAIR, T], BF)
        wqS = big.tile([128, NCH, 512], BF)
        wkS = big.tile([128, NCH, 512], BF)
        wvS = big.tile([128, NCH, 512], BF)
        woS = big.tile([128, 4, 1024], BF)
        csinS = big.tile([128, 4, 2, 512], BF)
        pkS = big.tile([128, 384], BF)
        signS = big.tile([128, 1], F32)
        pswapS = pkS[:, 0:128]
        maskS = pkS[:, 128:384]

        # ---- input DMAs: every tensor split in half across the two HWDGE
        # queues, emitted in strict first-use order so the first Q/V tiles
        # can start ~4us in instead of waiting behind monolithic loads ----
        nc.vector.memset(vOnes[:, :, :, 0:64], 1.0)

        def dma2(dst_lo, src_lo, dst_hi, src_hi):
            nc.scalar.dma_start(out=dst_lo, in_=src_lo)
            nc.sync.dma_start(out=dst_hi, in_=src_hi)

        def dma_w8(dst, src):  # [128, 8, 512] weight halves
            dma2(dst[:, 0:4, :], src[:, 0:2048], dst[:, 4:8, :], src[:, 2048:4096])

        def dma_csin(tch):
            c0 = tch * 1024
            dma2(
                csinS[:, tch, 0, :], csin_in[:, c0 : c0 + 512],
                csinS[:, tch, 1, :], csin_in[:, c0 + 512 : c0 + 1024],
            )

        def dma_xt(tch):
            c0 = tch * NCH * 512
            dma2(
                xT[:, tch, 0:4, :], xt_in[:, c0 : c0 + 2048],
                xT[:, tch, 4:8, :], xt_in[:, c0 + 2048 : c0 + 4096],
            )

        # first loads at 2-chunk granularity: chunk c's proj matmul can
        # start as soon as wq[c]+xt0[c] land instead of waiting 0.5MB halves
        for g in range(4):
            cs, ce = g * 1024, g * 1024 + 1024
            eng = nc.scalar if g % 2 == 0 else nc.sync
            eng.dma_start(out=wqS[:, 2 * g : 2 * g + 2, :], in_=wq_in[:, cs:ce])
            eng2 = nc.sync if g % 2 == 0 else nc.scalar
            eng2.dma_start(out=xT[:, 0, 2 * g : 2 * g + 2, :], in_=xt_in[:, cs:ce])
        nc.scalar.dma_start(out=signS, in_=sign_in)
        nc.sync.dma_start(out=pkS, in_=pk_in)
        dma_csin(0)
        dma_w8(wvS, wv_in)
        dma_xt(1)
        dma_csin(1)
        dma_csin(2)
        dma_w8(wkS, wk_in)
        dma_xt(2)
        dma_xt(3)
        dma_csin(3)
        dma2(woS[:, 0:2, :], wo_in[:, 0:2048], woS[:, 2:4, :], wo_in[:, 2048:4096])

        _evac_alt = [0]

        def _evac(out, in_):
            # alternate psum evacuations between ACT and DVE queues
            _evac_alt[0] ^= 1
            if _evac_alt[0]:
                nc.scalar.copy(out=out, in_=in_)
            else:
                nc.vector.tensor_copy(out, in_)

        # ---- projection tile helpers (split so the swap matmul can be
        # queued late, after other PE work, hiding the rope DVE latency) ----
        def proj_mm_r(wS, r, tch):
            ps = mmps.tile([128, 512], F32, tag="mm")
            for c in range(NCH):
                nc.tensor.matmul(
                    ps,
                    wS[:, c, r * 128 : r * 128 + 128],
                    xT[:, tch, c, :],
                    start=(c == 0),
                    stop=(c == NCH - 1),
                )
            return ps

        def rope_pre(ps, tch, evac_eng=None):
            pb = ab.tile([128, 512], BF, tag="pb")
            if evac_eng is None:
                _evac(pb, ps)
            elif evac_eng == "v":
                nc.vector.tensor_copy(pb, ps)
            w1 = ab.tile([128, 512], BF, tag="w1")
            t2 = ab.tile([128, 512], BF, tag="t2")
            nc.vector.tensor_mul(w1, pb, csinS[:, tch, 1, :])
            nc.vector.tensor_mul(t2, pb, csinS[:, tch, 0, :])
            return w1, t2

        def rope_swap(w1, t2, dest, r, tch, us_pool=None):
            tsl = slice(tch * 512, tch * 512 + 512)
            # phase 2 routes the swap psum to the (then-idle) pv pool so the
            # mmps ring holds one tile per proj and the PE never waits evac
            us = (us_pool or mmps).tile(
                [128, 512], F32, tag="pv" if us_pool is not None else "mm"
            )
            nc.tensor.matmul(us, pswapS, w1, start=True, stop=True)
            # rope = swap(P*sin) * sign + P*cos
            nc.vector.scalar_tensor_tensor(
                out=dest[:, r, tsl],
                in0=us,
                scalar=signS[:, 0:1],
                in1=t2,
                op0=OP.mult,
                op1=OP.add,
            )

        def v_tile(tt):
            tch, off = tt // 4, (tt % 4) * 128
            ps = mmps.tile([128, 512], F32, tag="mm")
            for c in range(NCH):
                nc.tensor.matmul(
                    ps,
                    xT[:, tch, c, off : off + 128],
                    wvS[:, c, :],
                    start=(c == 0),
                    stop=(c == NCH - 1),
                )
            _evac(vOnes[:, tt, :, 64:128], ps)

        # ---- phase 2: Q projection interleaved with V so the PE queue has
        # V work to fill Q's rope bubbles ----
        qlist = [(r, tch) for tch in range(4) for r in range(NPAIR)]  # 16
        for i in range(16):
            r, tch = qlist[i]
            ps = proj_mm_r(wqS, r, tch)
            w1, t2 = rope_pre(ps, tch)
            v_tile(i)
            rope_swap(w1, t2, qT, r, tch, us_pool=pvps)

        # ---- phase 3: K projection + attention, software-pipelined ----
        # sub-step si = 2p + sub. Scores of si interleave (in the PE queue)
        # with PV groups of si-1 and the next pair's K-projection chunks so
        # the PE keeps busy while ACT drains the per-block exp chain.
        e_tiles = {}

        def k_chunks(p):
            # 8 thunks: 4 mm chains and 4 swap finishes, swap_i after mm_i
            thunks = []
            pend = {}

            def mk_mm(tch):
                def f():
                    ps = proj_mm_r(wkS, p, tch)
                    pend[tch] = rope_pre(ps, tch, evac_eng="v")

                return f

            def mk_swap(tch):
                def f():
                    w1, t2 = pend.pop(tch)
                    rope_swap(w1, t2, kT, p, tch)

                return f

            order = [mk_mm(0), mk_mm(1), mk_swap(0), mk_mm(2), mk_swap(1),
                     mk_mm(3), mk_swap(2), mk_swap(3)]
            return order

        def sc_chunks(p, sub, si):
            rows = slice(64 * sub, 64 * sub + 64)

            def mk(b):
                def f():
                    ghi = min(b + 4, NBT - 1)
                    span = (ghi - b + 1) * 128
                    q0 = b * 128
                    sc = scps.tile([128, 640], F32, tag="sc")
                    for c0 in range(0, span, 512):
                        c1 = min(c0 + 512, span)
                        nc.tensor.matmul(
                            sc[:, c0:c1],
                            kT[rows, p, b * 128 : b * 128 + 128],
                            qT[rows, p, q0 + c0 : q0 + c1],
                            start=True,
                            stop=True,
                        )
                    et = epool.tile([128, 640], BF, tag="et")
                    nc.scalar.activation(
                        out=et[:, 0:span], in_=sc[:, 0:span], func=AF.Exp, scale=0.125
                    )
                    # boundary masks (multiplicative, post-exp) on gpsimd:
                    # causal at cols 0:128 (g=b), window at span-128 (g=b+4)
                    if b <= NBT - 5:
                        sel = _pair_cols(et[:, 0:640], 0, span - 128, 128)
                        nc.gpsimd.tensor_mul(sel, sel, maskS[:, 0:256])
                    else:
                        nc.gpsimd.tensor_mul(
                            et[:, 0:128], et[:, 0:128], maskS[:, 0:128]
                        )
                    e_tiles[(si, b)] = (et, q0, span)

                return f

            return [mk(b) for b in range(NBT)]

        def pv_chunks(p, sub, si):
            h = 2 * p + sub
            rows_h = slice(64 * sub, 64 * sub + 64)

            def mk(qh):
                def f():
                    qa0, qb0 = qh * 512, qh * 512 + 512
                    bstar = max(0, 4 * qh - 1)
                    blist = [bstar] + [
                        b
                        for b in range(NBT)
                        if b != bstar
                        and b * 128 < qb0
                        and (min(b + 4, NBT - 1) + 1) * 128 > qa0
                    ]
                    pv = pvps.tile([128, 512], F32, tag="pv")
                    for i, b in enumerate(blist):
                        et, q0, span = e_tiles[(si, b)]
                        ghi = min(b + 4, NBT - 1)
                        s0 = max(b * 128, qa0)
                        s1 = min((ghi + 1) * 128, qb0)
                        nc.tensor.matmul(
                            pv[:, s0 - qa0 : s1 - qa0],
                            vOnes[:, b, h, :],
                            et[:, s0 - q0 : s1 - q0],
                            start=(i == 0),
                            stop=(i == len(blist) - 1),
                        )
                    # normalize: attnT[rows_h] = O / sums
                    rec = rpool.tile([64, 512], F32, tag="rec")
                    nc.vector.reciprocal_approx_fast(out=rec, in_=pv[0:64, :])
                    nc.vector.tensor_mul(
                        attnT[rows_h, p, qa0:qb0], pv[64:128, :], rec
                    )

                return f

            return [mk(qh) for qh in range(4)]

        def zip_emit(primary, fillers):
            # emit primary thunks with fillers spliced in every 2 primaries
            fi = 0
            for i, th in enumerate(primary):
                th()
                if i % 2 == 1 and fi < len(fillers):
                    fillers[fi]()
                    fi += 1
            for th in fillers[fi:]:
                th()

        for th in k_chunks(0):
            th()
        prev_pv = None
        for p in range(NPAIR):
            kc = k_chunks(p + 1) if p < NPAIR - 1 else []
            for sub in range(2):
                si = 2 * p + sub
                fillers = []
                if prev_pv is not None:
                    fillers += prev_pv
                fillers += kc[:4] if sub == 0 else kc[4:]
                zip_emit(sc_chunks(p, sub, si), fillers)
                prev_pv = pv_chunks(p, sub, si)

        # ---- phase 4: output projection (partial: contraction over the 8
        # local heads; host sums the two head-group partials per batch),
        # interleaved with the last sub's PV groups ----
        def wo_tile(qt):
            st = stpool.tile([128, 1024], BF, tag="st")
            for nh in range(2):
                # nh1 psum lives in the (now idle) scores pool so the mmps
                # ring holds one tile per qt and the PE never waits evac
                if nh == 0:
                    ps = mmps.tile([128, 512], F32, tag="mm")
                else:
                    ps640 = scps.tile([128, 640], F32, tag="sc")
                    ps = ps640[:, 0:512]
                for c in range(NPAIR):
                    nc.tensor.matmul(
                        ps,
                        attnT[:, c, qt * 128 : qt * 128 + 128],
                        woS[:, c, nh * 512 : nh * 512 + 512],
                        start=(c == 0),
                        stop=(c == NPAIR - 1),
                    )
                _evac(st[:, nh * 512 : nh * 512 + 512], ps)
                eng = nc.sync if (qt + nh) % 2 == 0 else nc.scalar
                eng.dma_start(
                    out=out_d[qt * 128 : qt * 128 + 128, nh * 512 : nh * 512 + 512],
                    in_=st[:, nh * 512 : nh * 512 + 512],
                )

        for qh in range(4):
            prev_pv[qh]()
            for qt in range(4 * qh, 4 * qh + 4):
                wo_tile(qt)

        if debug_dumps:
            for nm, tl, sh in (
                ("d_xT", xT, [128, 4 * NCH * 512]),
                ("d_qT", qT, [128, NPAIR * T]),
                ("d_kT", kT, [128, NPAIR * T]),
                ("d_vOnes", vOnes, [128, NBT * 8 * 128]),
                ("d_attnT", attnT, [128, NPAIR * T]),
            ):
                dd = nc.dram_tensor(nm, sh, BF, kind="ExternalOutput").ap()
                nc.sync.dma_start(out=dd, in_=tl)

    nc.compile()
    return nc


def _host_inputs(x, token_positions, Wq, Wk, Wv, Wo):
    x = np.asarray(x, dtype=np.float32)
    pos = np.asarray(token_positions).astype(np.float64)
    Wq = np.asarray(Wq, np.float32)
    Wk = np.asarray(Wk, np.float32)
    Wv = np.asarray(Wv, np.float32)
    Wo = np.asarray(Wo, np.float32)

    invf = THETA ** (-np.arange(32, dtype=np.float64) / 32.0)
    ang = invf[:, None] * pos[None, :]  # [32, T]
    cos_t = np.tile(np.cos(ang), (4, 1)).astype(BF16)  # [128, T]
    sin_t = np.tile(np.sin(ang), (4, 1)).astype(BF16)
    # per-tch [cos(512) | sin(512)] chunks for just-in-time DMA
    csin = np.stack(
        [cos_t.reshape(128, 4, 512), sin_t.reshape(128, 4, 512)], axis=2
    )  # [128, 4, 2, 512]
    csin = np.ascontiguousarray(csin).reshape(128, 4096)
    perm = np.r_[32:64, 0:32, 96:128, 64:96]
    P = np.zeros((128, 128), np.float32)
    P[np.arange(128), perm] = 1.0
    pswapT = np.ascontiguousarray(P.T).astype(BF16)
    r = np.arange(128)[:, None]  # kv row (partition)
    c = np.arange(128)[None, :]  # q col (free)
    m_causal = (c >= r).astype(BF16)
    m_window = (c <= r).astype(BF16)
    pk = np.ascontiguousarray(
        np.concatenate([pswapT, m_causal, m_window], axis=1)
    )  # [128, 384]
    sign = np.tile(np.repeat(np.float32([-1, 1]), 32), 2).reshape(128, 1)

    def prep_qk(W, hg):
        Ws = W[hg * 512 : (hg + 1) * 512]  # [512 n, 1024 m]
        # eo-permute within each head: [64] -> [evens(32), odds(32)]
        Wp = Ws.reshape(8, 32, 2, 1024).transpose(0, 2, 1, 3).reshape(512, 1024)
        wt = Wp.T.reshape(NCH, 128, 512).transpose(1, 0, 2)  # [128, c, n']
        return np.ascontiguousarray(wt.astype(BF16)).reshape(128, NCH * 512)

    def prep_v(W, hg):
        Ws = W[hg * 512 : (hg + 1) * 512]
        wt = Ws.T.reshape(NCH, 128, 512).transpose(1, 0, 2)
        return np.ascontiguousarray(wt.astype(BF16)).reshape(128, NCH * 512)

    def prep_o(W, hg):
        WoC = W[:, hg * 512 : (hg + 1) * 512]  # [1024 n, 512 m]
        wt = WoC.T.reshape(4, 128, 1024).transpose(1, 0, 2)  # [128, c, n]
        return np.ascontiguousarray(wt.astype(BF16)).reshape(128, 4 * 1024)

    wq_hg = [prep_qk(Wq, hg) for hg in range(2)]
    wk_hg = [prep_qk(Wk, hg) for hg in range(2)]
    wv_hg = [prep_v(Wv, hg) for hg in range(2)]
    wo_hg = [prep_o(Wo, hg) for hg in range(2)]

    xt_b = []
    for b in range(B):
        xb = x[b].astype(BF16)  # [T, D]
        xt = xb.reshape(4, 512, NCH, 128).transpose(3, 0, 2, 1)  # [128,tch,c,j]
        xt_b.append(np.ascontiguousarray(xt).reshape(128, 4 * NCH * 512))

    in_maps = []
    for core in range(8):
        b, hg = divmod(core, 2)
        in_maps.append(
            {
                "xt": xt_b[b],
                "wq": wq_hg[hg],
                "wk": wk_hg[hg],
                "wv": wv_hg[hg],
                "wo": wo_hg[hg],
                "csin": csin,
                "pk": pk,
                "sign_t": sign,
            }
        )
    return in_maps


def _get_nc():
    if "nc" not in _CACHE:
        _CACHE["nc"] = _build()
    return _CACHE["nc"]


def kernel(x, token_positions, Wq, Wk, Wv, Wo, _trace=False):
    from concourse.bass_utils import run_bass_kernel_spmd

    nc = _get_nc()
    in_maps = _host_inputs(x, token_positions, Wq, Wk, Wv, Wo)
    res = run_bass_kernel_spmd(nc, in_maps, core_ids=list(range(8)), trace=_trace)
    _CACHE["last_result"] = res
    out = np.zeros((B, T, D), np.float32)
    for b in range(B):
        out[b] = res.results[2 * b]["out"].astype(np.float32) + res.results[
            2 * b + 1
        ]["out"].astype(np.float32)
    return out
